# revision 28
# baseline (speedup 1.0000x reference)
"""Trainium2 Bass kernel for batched multi-head graph attention (GAT).

Reference computation (per batch b, head h):
    h_prime = h[b] @ w[h]                      # [N, FOUT]
    t = tanh(h_prime)
    src = t @ a_src[h]; dst = t @ a_dst[h]     # [N]
    s[i,j] = leaky_relu_{0.2}(src[i] + dst[j])
    attn = softmax_j(where(adj[b]>0, s, -inf))
    out[b,h] = attn @ h_prime

Device algorithm (core c <-> batch b=c):
    exp(leaky_relu(s)) = max(e^s, e^{0.2 s}), and with s = src_i + dst_j the
    unnormalized weight factors as
        w[j,i] = adjT[j,i] * e^{src_i} * q_j * max(1, u_i * v_j)
    with u = e^{-0.8 src}, v = e^{-0.8 dst}, q = e^{dst}. The e^{src_i} factor
    is shared by numerator and denominator of the softmax, so it cancels and is
    never computed. q_j folds into the matmul's stationary operand
    hp_q = [h_prime * q | q]; its 65th (ones*q) column accumulates the softmax
    denominator. Per 128-row chunk of the score matrix only two DVE ops run:
        mx = tensor_scalar(uB, *v_j, max 1.0)   (4x rate, bf16)
        Z  = tensor_tensor(mx, adjT, mult)      (2x rate, bf16)
    and the PE contracts outT[f,i] += hp_q[j,f] * Z[j,i]. The host divides
    rows 0..63 by row 64 and transposes to [b, h, n, f].
"""

import numpy as np
import ml_dtypes

import concourse.bass as bass
import concourse.mybir as mybir
import concourse.tile as tile
from concourse import bacc
from concourse.bass_utils import run_bass_kernel_spmd
BS, N, FIN, NH, FOUT = 8, 1024, 256, 8, 64
P = 128
NCH = N // P          # 8 chunks of the node axis
KC = FIN // P         # 2 chunks of the feature-in axis
F32 = mybir.dt.float32
F32R = mybir.dt.float32r
BF16 = mybir.dt.bfloat16
AX = mybir.AxisListType
ALU = mybir.AluOpType
ACTF = mybir.ActivationFunctionType
BF16NP = ml_dtypes.bfloat16

def emit(nc, tc, hT_d, w_d, aB_d, adjT_d, ident_d, out_d):
    with (
        tc.tile_pool(name="const", bufs=1) as cpool,
        tc.tile_pool(name="t", bufs=2) as tpool,
        tc.tile_pool(name="tmp", bufs=2) as mpool,
        tc.tile_pool(name="ub", bufs=3) as ubpool,
        tc.tile_pool(name="mx", bufs=6) as mxpool,
        tc.tile_pool(name="z", bufs=6) as zpool,
        tc.tile_pool(name="osb", bufs=2) as opool,
        tc.tile_pool(name="pshp", bufs=2, space="PSUM") as pp_hp,
        tc.tile_pool(name="psut", bufs=2, space="PSUM") as pp_ut,
        tc.tile_pool(name="psout", bufs=4, space="PSUM") as pp_out,
    ):
        # ---- constant loads ----
        hT = cpool.tile([P, KC, N], BF16)
        wsb = cpool.tile([P, KC, NH * FOUT], BF16)
        aB = cpool.tile([P, 2, NH * FOUT], BF16)
        adjT = cpool.tile([P, NCH, N], BF16)
        for kc in range(KC):
            for hf in range(2):
                nc.sync.dma_start(
                    hT[:, kc, hf * 512 : (hf + 1) * 512],
                    hT_d[kc, :, hf * 512 : (hf + 1) * 512],
                )
            nc.scalar.dma_start(wsb[:, kc, :], w_d[kc])
        for i in range(2):
            nc.scalar.dma_start(aB[:, i, :], aB_d[i])
        for jc in range(NCH):
            (nc.scalar if jc % 2 else nc.sync).dma_start(adjT[:, jc, :], adjT_d[jc])
        ident = cpool.tile([P, P], F32)
        nc.sync.dma_start(ident, ident_d)
        ones_row = cpool.tile([1, P], BF16)
        nc.vector.memset(ones_row, 1.0)

        hp_sb = cpool.tile([P, NCH, NH, FOUT + 1], BF16)
        nc.vector.memset(hp_sb[:, :, :, FOUT : FOUT + 1], 1.0)
        # sd_col[:, ic, 0, :] = src projection, [:, ic, 1, :] = dst projection
        sd_col = cpool.tile([P, NCH, 2, NH], F32)
        u_rows = cpool.tile([NH, N], BF16)
        uts = []

        # ---- phase A: h_prime, tanh, projections ----
        for ic in range(NCH):
            ps = pp_hp.tile([P, NH * FOUT], F32, tag="hp")
            for kc in range(KC):
                nc.tensor.matmul(
                    ps,
                    hT[:, kc, ic * P : (ic + 1) * P],
                    wsb[:, kc, :],
                    start=(kc == 0),
                    stop=(kc == KC - 1),
                )
            t = tpool.tile([P, NH, FOUT], BF16)
            nc.scalar.activation(
                t, ps.rearrange("p (h f) -> p h f", f=FOUT), ACTF.Tanh
            )
            nc.scalar.activation(
                hp_sb[:, ic, :, 0:FOUT],
                ps.rearrange("p (h f) -> p h f", f=FOUT),
                ACTF.Copy,
            )
            tm = mpool.tile([P, 2, NH, FOUT], BF16, tag="proj")
            nc.vector.tensor_tensor(
                tm,
                aB.rearrange("p w (h f) -> p w h f", f=FOUT),
                t[:, None, :, :].to_broadcast([P, 2, NH, FOUT]),
                ALU.mult,
            )
            nc.vector.tensor_reduce(sd_col[:, ic, :, :], tm, AX.X, ALU.add)
            if ic % 4 == 0:
                ut = pp_ut.tile([NH, 4 * P], F32, tag="ut", name=f"ut{ic // 4}")
                uts.append(ut)
            nc.tensor.transpose(
                uts[-1][:, (ic % 4) * P : (ic % 4 + 1) * P], sd_col[:, ic, 0, :], ident
            )
            if ic % 4 == 3:
                nc.scalar.activation(
                    u_rows[:, (ic - 3) * P : (ic + 1) * P],
                    uts[-1],
                    ACTF.Exp,
                    scale=-0.8,
                )

        # ---- phase B: exponentials ----
        q_col = cpool.tile([P, NCH, NH], F32)
        v_col = cpool.tile([P, NCH, NH], F32)
        nc.scalar.activation(q_col, sd_col[:, :, 1, :], ACTF.Exp)
        nc.scalar.activation(v_col, sd_col[:, :, 1, :], ACTF.Exp, scale=-0.8)

        # ---- phase C: masked weights + attention matmuls ----
        hp_q = cpool.tile([P, NH, NCH, FOUT + 1], BF16)
        for h in range(NH):
            for ic in range(NCH):
                if h == 0:
                    nc.vector.tensor_scalar(
                        hp_q[:, h, ic, :],
                        hp_sb[:, ic, h, :],
                        q_col[:, ic, h : h + 1],
                        None,
                        ALU.mult,
                    )
                else:
                    nc.scalar.activation(
                        hp_q[:, h, ic, :],
                        hp_sb[:, ic, h, :],
                        ACTF.Copy,
                        scale=q_col[:, ic, h : h + 1],
                    )
            stage = ubpool.tile([1, N], BF16, tag="stage")
            nc.sync.dma_start(stage, u_rows[h : h + 1, :])
            ub = ubpool.tile([P, N], BF16)
            for half in range(2):
                ubps = pp_ut.tile([P, 512], F32, tag="ut", name=f"ubps{half}")
                nc.tensor.matmul(
                    ubps, ones_row, stage[:, half * 512 : (half + 1) * 512],
                    start=True, stop=True,
                )
                nc.scalar.activation(
                    ub[:, half * 512 : (half + 1) * 512], ubps, ACTF.Copy
                )
            pso = [
                pp_out.tile([FOUT + 1, 512], F32, tag="out", name=f"pso{half}")
                for half in range(2)
            ]
            for jc2 in range(NCH // 2):
                mx = mxpool.tile([P, 2, N], BF16)
                for k in range(2):
                    jc = 2 * jc2 + k
                    nc.vector.tensor_scalar(
                        mx[:, k, :], ub, v_col[:, jc, h : h + 1], 1.0, ALU.mult, ALU.max
                    )
                z = zpool.tile([P, 2, N], BF16)
                nc.vector.tensor_tensor(
                    z, mx, adjT[:, 2 * jc2 : 2 * jc2 + 2, :], ALU.mult
                )
                for k in range(2):
                    jc = 2 * jc2 + k
                    for half in range(2):
                        nc.tensor.matmul(
                            pso[half],
                            hp_q[:, h, jc, :],
                            z[:, k, half * 512 : (half + 1) * 512],
                            start=(jc == 0),
                            stop=(jc == NCH - 1),
                        )
            ot = opool.tile([FOUT + 1, N], F32)
            nc.scalar.activation(ot[:, 0:512], pso[0], ACTF.Copy)
            nc.scalar.activation(ot[:, 512:N], pso[1], ACTF.Copy)
            nc.sync.dma_start(out_d[h], ot)


def build_program(num_devices=8, debug=False):
    nc = bacc.Bacc(
        "TRN2", target_bir_lowering=False, debug=debug, num_devices=num_devices
    )
    hT_d = nc.dram_tensor("hT", [KC, P, N], BF16, kind="ExternalInput").ap()
    w_d = nc.dram_tensor("w_all", [KC, P, NH * FOUT], BF16, kind="ExternalInput").ap()
    aB_d = nc.dram_tensor("aB", [2, P, NH * FOUT], BF16, kind="ExternalInput").ap()
    adjT_d = nc.dram_tensor("adjT", [NCH, P, N], BF16, kind="ExternalInput").ap()
    ident_d = nc.dram_tensor("ident", [P, P], F32, kind="ExternalInput").ap()
    out_d = nc.dram_tensor("outT", [NH, FOUT + 1, N], F32, kind="ExternalOutput").ap()
    with tile.TileContext(nc) as tc:
        emit(nc, tc, hT_d, w_d, aB_d, adjT_d, ident_d, out_d)
    nc.compile()
    return nc


def make_in_maps(h, adj, w, a_src, a_dst):
    """Host-side sharding/layout prep: core c gets batch c."""
    w_all = np.ascontiguousarray(
        w.astype(np.float32).transpose(1, 0, 2).reshape(KC, P, NH * FOUT)
    ).astype(BF16NP)
    a_cat = np.stack(
        [a_src[..., 0].reshape(NH * FOUT), a_dst[..., 0].reshape(NH * FOUT)]
    )
    aB = np.ascontiguousarray(
        np.broadcast_to(a_cat[:, None, :], (2, P, NH * FOUT))
    ).astype(BF16NP)
    ident = np.eye(P, dtype=np.float32)
    in_maps = []
    for b in range(BS):
        hT = np.ascontiguousarray(h[b].astype(np.float32).T.reshape(KC, P, N)).astype(BF16NP)
        adjT = np.ascontiguousarray(adj[b].T.reshape(NCH, P, N)).astype(BF16NP)
        in_maps.append(
            {"hT": hT, "w_all": w_all, "aB": aB, "adjT": adjT, "ident": ident}
        )
    return in_maps


def postprocess(raw_outs):
    """raw_outs: list of [NH, FOUT+1, N] per core -> full [BS, NH, N, FOUT]."""
    outT = np.stack(raw_outs)  # [BS, NH, FOUT+1, N]
    num = outT[:, :, 0:FOUT, :]
    den = outT[:, :, FOUT : FOUT + 1, :]
    return np.ascontiguousarray((num / den).transpose(0, 1, 3, 2)).astype(np.float32)


_NC_CACHE = {}


def kernel(h, adj, w, a_src, a_dst):
    if "nc" not in _NC_CACHE:
        _NC_CACHE["nc"] = build_program(num_devices=BS)
    nc = _NC_CACHE["nc"]
    in_maps = make_in_maps(h, adj, w, a_src, a_dst)
    res = run_bass_kernel_spmd(nc, in_maps, core_ids=list(range(BS)))
    return postprocess([r["outT"] for r in res.results])


# revision 29
# speedup vs baseline: 1.0313x; 1.0313x over previous
"""Trainium2 Bass kernel for batched multi-head graph attention (GAT).

Reference computation (per batch b, head h):
    h_prime = h[b] @ w[h]                      # [N, FOUT]
    t = tanh(h_prime)
    src = t @ a_src[h]; dst = t @ a_dst[h]     # [N]
    s[i,j] = leaky_relu_{0.2}(src[i] + dst[j])
    attn = softmax_j(where(adj[b]>0, s, -inf))
    out[b,h] = attn @ h_prime

Device algorithm (core c <-> batch b=c):
    exp(leaky_relu(s)) = max(e^s, e^{0.2 s}), and with s = src_i + dst_j the
    unnormalized weight factors as
        w[j,i] = adjT[j,i] * e^{src_i} * q_j * max(1, u_i * v_j)
    with u = e^{-0.8 src}, v = e^{-0.8 dst}, q = e^{dst}. The e^{src_i} factor
    is shared by numerator and denominator of the softmax, so it cancels and is
    never computed. q_j folds into the matmul's stationary operand
    hp_q = [h_prime * q | q]; its 65th (ones*q) column accumulates the softmax
    denominator. Per 128-row chunk of the score matrix only two DVE ops run:
        mx = tensor_scalar(uB, *v_j, max 1.0)   (4x rate, bf16)
        Z  = tensor_tensor(mx, adjT, mult)      (2x rate, bf16)
    and the PE contracts outT[f,i] += hp_q[j,f] * Z[j,i]. The host divides
    rows 0..63 by row 64 and transposes to [b, h, n, f].
"""

import numpy as np
import ml_dtypes

import concourse.bass as bass
import concourse.mybir as mybir
import concourse.tile as tile
from concourse import bacc
from concourse.bass_utils import run_bass_kernel_spmd
BS, N, FIN, NH, FOUT = 8, 1024, 256, 8, 64
P = 128
NCH = N // P          # 8 chunks of the node axis
KC = FIN // P         # 2 chunks of the feature-in axis
F32 = mybir.dt.float32
F32R = mybir.dt.float32r
BF16 = mybir.dt.bfloat16
AX = mybir.AxisListType
ALU = mybir.AluOpType
ACTF = mybir.ActivationFunctionType
BF16NP = ml_dtypes.bfloat16

def emit(nc, tc, hT_d, w_d, aB_d, adjT_d, ident_d, out_d):
    with (
        tc.tile_pool(name="const", bufs=1) as cpool,
        tc.tile_pool(name="t", bufs=2) as tpool,
        tc.tile_pool(name="tmp", bufs=2) as mpool,
        tc.tile_pool(name="ub", bufs=3) as ubpool,
        tc.tile_pool(name="mx", bufs=6) as mxpool,
        tc.tile_pool(name="z", bufs=6) as zpool,
        tc.tile_pool(name="osb", bufs=2) as opool,
        tc.tile_pool(name="pshp", bufs=2, space="PSUM") as pp_hp,
        tc.tile_pool(name="psut", bufs=2, space="PSUM") as pp_ut,
        tc.tile_pool(name="psout", bufs=4, space="PSUM") as pp_out,
    ):
        # ---- constant loads ----
        hT = cpool.tile([P, KC, N], BF16)
        wsb = cpool.tile([P, KC, NH * FOUT], BF16)
        aB = cpool.tile([P, 2, NH * FOUT], BF16)
        adjT = cpool.tile([P, NCH, N], BF16)
        for kc in range(KC):
            for hf in range(2):
                nc.sync.dma_start(
                    hT[:, kc, hf * 512 : (hf + 1) * 512],
                    hT_d[kc, :, hf * 512 : (hf + 1) * 512],
                )
            nc.scalar.dma_start(wsb[:, kc, :], w_d[kc])
        for i in range(2):
            nc.scalar.dma_start(aB[:, i, :], aB_d[i])
        for jc in range(NCH):
            (nc.scalar if jc % 2 else nc.sync).dma_start(adjT[:, jc, :], adjT_d[jc])
        ident = cpool.tile([P, P], F32)
        nc.sync.dma_start(ident, ident_d)
        ones_row = cpool.tile([1, P], BF16)
        nc.vector.memset(ones_row, 1.0)

        hp_sb = cpool.tile([P, NCH, NH, FOUT + 1], BF16)
        nc.vector.memset(hp_sb[:, :, :, FOUT : FOUT + 1], 1.0)
        # sd_col[:, ic, 0, :] = src projection, [:, ic, 1, :] = dst projection
        sd_col = cpool.tile([P, NCH, 2, NH], F32)
        u_rows = cpool.tile([NH, N], BF16)
        uts = []

        # ---- phase A: h_prime, tanh, projections ----
        for ic in range(NCH):
            ps = pp_hp.tile([P, NH * FOUT], F32, tag="hp")
            for kc in range(KC):
                nc.tensor.matmul(
                    ps,
                    hT[:, kc, ic * P : (ic + 1) * P],
                    wsb[:, kc, :],
                    start=(kc == 0),
                    stop=(kc == KC - 1),
                )
            t = tpool.tile([P, NH, FOUT], BF16)
            nc.scalar.activation(
                t, ps.rearrange("p (h f) -> p h f", f=FOUT), ACTF.Tanh
            )
            nc.scalar.activation(
                hp_sb[:, ic, :, 0:FOUT],
                ps.rearrange("p (h f) -> p h f", f=FOUT),
                ACTF.Copy,
            )
            tm = mpool.tile([P, 2, NH, FOUT], BF16, tag="proj")
            nc.vector.tensor_tensor(
                tm,
                aB.rearrange("p w (h f) -> p w h f", f=FOUT),
                t[:, None, :, :].to_broadcast([P, 2, NH, FOUT]),
                ALU.mult,
            )
            nc.vector.tensor_reduce(sd_col[:, ic, :, :], tm, AX.X, ALU.add)
            if ic % 4 == 0:
                ut = pp_ut.tile([NH, 4 * P], F32, tag="ut", name=f"ut{ic // 4}")
                uts.append(ut)
            nc.tensor.transpose(
                uts[-1][:, (ic % 4) * P : (ic % 4 + 1) * P], sd_col[:, ic, 0, :], ident
            )
            if ic % 4 == 3:
                nc.scalar.activation(
                    u_rows[:, (ic - 3) * P : (ic + 1) * P],
                    uts[-1],
                    ACTF.Exp,
                    scale=-0.8,
                )

        # ---- phase B: exponentials ----
        q_col = cpool.tile([P, NCH, NH], F32)
        v_col = cpool.tile([P, NCH, NH], F32)
        nc.scalar.activation(q_col, sd_col[:, :, 1, :], ACTF.Exp)
        nc.scalar.activation(v_col, sd_col[:, :, 1, :], ACTF.Exp, scale=-0.8)

        # ---- phase C: masked weights + attention matmuls ----
        hp_q = cpool.tile([P, NH, NCH, FOUT + 1], BF16)
        for h in range(NH):
            for ic in range(NCH):
                if h == 0:
                    nc.vector.tensor_scalar(
                        hp_q[:, h, ic, :],
                        hp_sb[:, ic, h, :],
                        q_col[:, ic, h : h + 1],
                        None,
                        ALU.mult,
                    )
                else:
                    nc.scalar.activation(
                        hp_q[:, h, ic, :],
                        hp_sb[:, ic, h, :],
                        ACTF.Copy,
                        scale=q_col[:, ic, h : h + 1],
                    )
            if h == 0:
                row = u_rows[0:1, :]
            else:
                stage = ubpool.tile([1, N], BF16, tag="stage")
                nc.sync.dma_start(stage, u_rows[h : h + 1, :])
                row = stage
            ub = ubpool.tile([P, N], BF16)
            for half in range(2):
                ubps = pp_ut.tile([P, 512], F32, tag="ut", name=f"ubps{half}")
                nc.tensor.matmul(
                    ubps, ones_row, row[:, half * 512 : (half + 1) * 512],
                    start=True, stop=True,
                )
                if h == 0:
                    nc.vector.tensor_copy(ub[:, half * 512 : (half + 1) * 512], ubps)
                else:
                    nc.scalar.activation(
                        ub[:, half * 512 : (half + 1) * 512], ubps, ACTF.Copy
                    )
            pso = [
                pp_out.tile([FOUT + 1, 512], F32, tag="out", name=f"pso{half}")
                for half in range(2)
            ]
            for jc2 in range(NCH // 2):
                mx = mxpool.tile([P, 2, N], BF16)
                for k in range(2):
                    jc = 2 * jc2 + k
                    nc.vector.tensor_scalar(
                        mx[:, k, :], ub, v_col[:, jc, h : h + 1], 1.0, ALU.mult, ALU.max
                    )
                z = zpool.tile([P, 2, N], BF16)
                nc.vector.tensor_tensor(
                    z, mx, adjT[:, 2 * jc2 : 2 * jc2 + 2, :], ALU.mult
                )
                for k in range(2):
                    jc = 2 * jc2 + k
                    for half in range(2):
                        nc.tensor.matmul(
                            pso[half],
                            hp_q[:, h, jc, :],
                            z[:, k, half * 512 : (half + 1) * 512],
                            start=(jc == 0),
                            stop=(jc == NCH - 1),
                        )
            ot = opool.tile([FOUT + 1, N], F32)
            nc.scalar.activation(ot[:, 0:512], pso[0], ACTF.Copy)
            nc.scalar.activation(ot[:, 512:N], pso[1], ACTF.Copy)
            nc.sync.dma_start(out_d[h], ot)


def build_program(num_devices=8, debug=False):
    nc = bacc.Bacc(
        "TRN2", target_bir_lowering=False, debug=debug, num_devices=num_devices
    )
    hT_d = nc.dram_tensor("hT", [KC, P, N], BF16, kind="ExternalInput").ap()
    w_d = nc.dram_tensor("w_all", [KC, P, NH * FOUT], BF16, kind="ExternalInput").ap()
    aB_d = nc.dram_tensor("aB", [2, P, NH * FOUT], BF16, kind="ExternalInput").ap()
    adjT_d = nc.dram_tensor("adjT", [NCH, P, N], BF16, kind="ExternalInput").ap()
    ident_d = nc.dram_tensor("ident", [P, P], F32, kind="ExternalInput").ap()
    out_d = nc.dram_tensor("outT", [NH, FOUT + 1, N], F32, kind="ExternalOutput").ap()
    with tile.TileContext(nc) as tc:
        emit(nc, tc, hT_d, w_d, aB_d, adjT_d, ident_d, out_d)
    nc.compile()
    return nc


def make_in_maps(h, adj, w, a_src, a_dst):
    """Host-side sharding/layout prep: core c gets batch c."""
    w_all = np.ascontiguousarray(
        w.astype(np.float32).transpose(1, 0, 2).reshape(KC, P, NH * FOUT)
    ).astype(BF16NP)
    a_cat = np.stack(
        [a_src[..., 0].reshape(NH * FOUT), a_dst[..., 0].reshape(NH * FOUT)]
    )
    aB = np.ascontiguousarray(
        np.broadcast_to(a_cat[:, None, :], (2, P, NH * FOUT))
    ).astype(BF16NP)
    ident = np.eye(P, dtype=np.float32)
    in_maps = []
    for b in range(BS):
        hT = np.ascontiguousarray(h[b].astype(np.float32).T.reshape(KC, P, N)).astype(BF16NP)
        adjT = np.ascontiguousarray(adj[b].T.reshape(NCH, P, N)).astype(BF16NP)
        in_maps.append(
            {"hT": hT, "w_all": w_all, "aB": aB, "adjT": adjT, "ident": ident}
        )
    return in_maps


def postprocess(raw_outs):
    """raw_outs: list of [NH, FOUT+1, N] per core -> full [BS, NH, N, FOUT]."""
    outT = np.stack(raw_outs)  # [BS, NH, FOUT+1, N]
    num = outT[:, :, 0:FOUT, :]
    den = outT[:, :, FOUT : FOUT + 1, :]
    return np.ascontiguousarray((num / den).transpose(0, 1, 3, 2)).astype(np.float32)


_NC_CACHE = {}


def kernel(h, adj, w, a_src, a_dst):
    if "nc" not in _NC_CACHE:
        _NC_CACHE["nc"] = build_program(num_devices=BS)
    nc = _NC_CACHE["nc"]
    in_maps = make_in_maps(h, adj, w, a_src, a_dst)
    res = run_bass_kernel_spmd(nc, in_maps, core_ids=list(range(BS)))
    return postprocess([r["outT"] for r in res.results])


# revision 30
# speedup vs baseline: 1.0314x; 1.0001x over previous
"""Trainium2 Bass kernel for batched multi-head graph attention (GAT).

Reference computation (per batch b, head h):
    h_prime = h[b] @ w[h]                      # [N, FOUT]
    t = tanh(h_prime)
    src = t @ a_src[h]; dst = t @ a_dst[h]     # [N]
    s[i,j] = leaky_relu_{0.2}(src[i] + dst[j])
    attn = softmax_j(where(adj[b]>0, s, -inf))
    out[b,h] = attn @ h_prime

Device algorithm (core c <-> batch b=c):
    exp(leaky_relu(s)) = max(e^s, e^{0.2 s}), and with s = src_i + dst_j the
    unnormalized weight factors as
        w[j,i] = adjT[j,i] * e^{src_i} * q_j * max(1, u_i * v_j)
    with u = e^{-0.8 src}, v = e^{-0.8 dst}, q = e^{dst}. The e^{src_i} factor
    is shared by numerator and denominator of the softmax, so it cancels and is
    never computed. q_j folds into the matmul's stationary operand
    hp_q = [h_prime * q | q]; its 65th (ones*q) column accumulates the softmax
    denominator. Per 128-row chunk of the score matrix only two DVE ops run:
        mx = tensor_scalar(uB, *v_j, max 1.0)   (4x rate, bf16)
        Z  = tensor_tensor(mx, adjT, mult)      (2x rate, bf16)
    and the PE contracts outT[f,i] += hp_q[j,f] * Z[j,i]. The host divides
    rows 0..63 by row 64 and transposes to [b, h, n, f].
"""

import numpy as np
import ml_dtypes

import concourse.bass as bass
import concourse.mybir as mybir
import concourse.tile as tile
from concourse import bacc
from concourse.bass_utils import run_bass_kernel_spmd
BS, N, FIN, NH, FOUT = 8, 1024, 256, 8, 64
P = 128
NCH = N // P          # 8 chunks of the node axis
KC = FIN // P         # 2 chunks of the feature-in axis
F32 = mybir.dt.float32
F32R = mybir.dt.float32r
BF16 = mybir.dt.bfloat16
AX = mybir.AxisListType
ALU = mybir.AluOpType
ACTF = mybir.ActivationFunctionType
BF16NP = ml_dtypes.bfloat16

def emit(nc, tc, hT_d, w_d, aB_d, adjT_d, ident_d, out_d):
    with (
        tc.tile_pool(name="const", bufs=1) as cpool,
        tc.tile_pool(name="t", bufs=3) as tpool,
        tc.tile_pool(name="tmp", bufs=3) as mpool,
        tc.tile_pool(name="ub", bufs=3) as ubpool,
        tc.tile_pool(name="mx", bufs=6) as mxpool,
        tc.tile_pool(name="z", bufs=6) as zpool,
        tc.tile_pool(name="osb", bufs=2) as opool,
        tc.tile_pool(name="pshp", bufs=2, space="PSUM") as pp_hp,
        tc.tile_pool(name="psut", bufs=2, space="PSUM") as pp_ut,
        tc.tile_pool(name="psout", bufs=4, space="PSUM") as pp_out,
    ):
        # ---- constant loads ----
        hT = cpool.tile([P, KC, N], BF16)
        wsb = cpool.tile([P, KC, NH * FOUT], BF16)
        aB = cpool.tile([P, 2, NH * FOUT], BF16)
        adjT = cpool.tile([P, NCH, N], BF16)
        for kc in range(KC):
            for hf in range(2):
                nc.sync.dma_start(
                    hT[:, kc, hf * 512 : (hf + 1) * 512],
                    hT_d[kc, :, hf * 512 : (hf + 1) * 512],
                )
            nc.scalar.dma_start(wsb[:, kc, :], w_d[kc])
        for i in range(2):
            nc.scalar.dma_start(aB[:, i, :], aB_d[i])
        for jc in range(NCH):
            (nc.scalar if jc % 2 else nc.sync).dma_start(adjT[:, jc, :], adjT_d[jc])
        ident = cpool.tile([P, P], F32)
        nc.sync.dma_start(ident, ident_d)
        ones_row = cpool.tile([1, P], BF16)
        nc.vector.memset(ones_row, 1.0)

        hp_sb = cpool.tile([P, NCH, NH, FOUT + 1], BF16)
        nc.vector.memset(hp_sb[:, :, :, FOUT : FOUT + 1], 1.0)
        # sd_col[:, ic, 0, :] = src projection, [:, ic, 1, :] = dst projection
        sd_col = cpool.tile([P, NCH, 2, NH], F32)
        u_rows = cpool.tile([NH, N], BF16)
        uts = []

        # ---- phase A: h_prime, tanh, projections ----
        for ic in range(NCH):
            ps = pp_hp.tile([P, NH * FOUT], F32, tag="hp")
            for kc in range(KC):
                nc.tensor.matmul(
                    ps,
                    hT[:, kc, ic * P : (ic + 1) * P],
                    wsb[:, kc, :],
                    start=(kc == 0),
                    stop=(kc == KC - 1),
                )
            t = tpool.tile([P, NH, FOUT], BF16)
            nc.scalar.activation(
                t, ps.rearrange("p (h f) -> p h f", f=FOUT), ACTF.Tanh
            )
            nc.scalar.activation(
                hp_sb[:, ic, :, 0:FOUT],
                ps.rearrange("p (h f) -> p h f", f=FOUT),
                ACTF.Copy,
            )
            tm = mpool.tile([P, 2, NH, FOUT], BF16, tag="proj")
            nc.vector.tensor_tensor(
                tm,
                aB.rearrange("p w (h f) -> p w h f", f=FOUT),
                t[:, None, :, :].to_broadcast([P, 2, NH, FOUT]),
                ALU.mult,
            )
            nc.vector.tensor_reduce(sd_col[:, ic, :, :], tm, AX.X, ALU.add)
            if ic % 4 == 0:
                ut = pp_ut.tile([NH, 4 * P], F32, tag="ut", name=f"ut{ic // 4}")
                uts.append(ut)
            nc.tensor.transpose(
                uts[-1][:, (ic % 4) * P : (ic % 4 + 1) * P], sd_col[:, ic, 0, :], ident
            )
            if ic % 4 == 3:
                nc.scalar.activation(
                    u_rows[:, (ic - 3) * P : (ic + 1) * P],
                    uts[-1],
                    ACTF.Exp,
                    scale=-0.8,
                )

        # ---- phase B: exponentials ----
        q_col = cpool.tile([P, NCH, NH], F32)
        v_col = cpool.tile([P, NCH, NH], F32)
        nc.scalar.activation(q_col, sd_col[:, :, 1, :], ACTF.Exp)
        nc.scalar.activation(v_col, sd_col[:, :, 1, :], ACTF.Exp, scale=-0.8)

        # ---- phase C: masked weights + attention matmuls ----
        hp_q = cpool.tile([P, NH, NCH, FOUT + 1], BF16)
        for h in range(NH):
            for ic in range(NCH):
                if h == 0:
                    nc.vector.tensor_scalar(
                        hp_q[:, h, ic, :],
                        hp_sb[:, ic, h, :],
                        q_col[:, ic, h : h + 1],
                        None,
                        ALU.mult,
                    )
                else:
                    nc.scalar.activation(
                        hp_q[:, h, ic, :],
                        hp_sb[:, ic, h, :],
                        ACTF.Copy,
                        scale=q_col[:, ic, h : h + 1],
                    )
            if h == 0:
                row = u_rows[0:1, :]
            else:
                stage = ubpool.tile([1, N], BF16, tag="stage")
                nc.scalar.dma_start(stage, u_rows[h : h + 1, :])
                row = stage
            ub = ubpool.tile([P, N], BF16)
            for half in range(2):
                ubps = pp_ut.tile([P, 512], F32, tag="ut", name=f"ubps{half}")
                nc.tensor.matmul(
                    ubps, ones_row, row[:, half * 512 : (half + 1) * 512],
                    start=True, stop=True,
                )
                if h == 0:
                    nc.vector.tensor_copy(ub[:, half * 512 : (half + 1) * 512], ubps)
                else:
                    nc.scalar.activation(
                        ub[:, half * 512 : (half + 1) * 512], ubps, ACTF.Copy
                    )
            pso = [
                pp_out.tile([FOUT + 1, 512], F32, tag="out", name=f"pso{half}")
                for half in range(2)
            ]
            for jc2 in range(NCH // 2):
                mx = mxpool.tile([P, 2, N], BF16)
                for k in range(2):
                    jc = 2 * jc2 + k
                    nc.vector.tensor_scalar(
                        mx[:, k, :], ub, v_col[:, jc, h : h + 1], 1.0, ALU.mult, ALU.max
                    )
                z = zpool.tile([P, 2, N], BF16)
                nc.vector.tensor_tensor(
                    z, mx, adjT[:, 2 * jc2 : 2 * jc2 + 2, :], ALU.mult
                )
                for k in range(2):
                    jc = 2 * jc2 + k
                    for half in range(2):
                        nc.tensor.matmul(
                            pso[half],
                            hp_q[:, h, jc, :],
                            z[:, k, half * 512 : (half + 1) * 512],
                            start=(jc == 0),
                            stop=(jc == NCH - 1),
                        )
            ot = opool.tile([FOUT + 1, N], F32)
            nc.scalar.activation(ot[:, 0:512], pso[0], ACTF.Copy)
            nc.scalar.activation(ot[:, 512:N], pso[1], ACTF.Copy)
            nc.sync.dma_start(out_d[h], ot)


def build_program(num_devices=8, debug=False):
    nc = bacc.Bacc(
        "TRN2", target_bir_lowering=False, debug=debug, num_devices=num_devices
    )
    hT_d = nc.dram_tensor("hT", [KC, P, N], BF16, kind="ExternalInput").ap()
    w_d = nc.dram_tensor("w_all", [KC, P, NH * FOUT], BF16, kind="ExternalInput").ap()
    aB_d = nc.dram_tensor("aB", [2, P, NH * FOUT], BF16, kind="ExternalInput").ap()
    adjT_d = nc.dram_tensor("adjT", [NCH, P, N], BF16, kind="ExternalInput").ap()
    ident_d = nc.dram_tensor("ident", [P, P], F32, kind="ExternalInput").ap()
    out_d = nc.dram_tensor("outT", [NH, FOUT + 1, N], F32, kind="ExternalOutput").ap()
    with tile.TileContext(nc) as tc:
        emit(nc, tc, hT_d, w_d, aB_d, adjT_d, ident_d, out_d)
    nc.compile()
    return nc


def make_in_maps(h, adj, w, a_src, a_dst):
    """Host-side sharding/layout prep: core c gets batch c."""
    w_all = np.ascontiguousarray(
        w.astype(np.float32).transpose(1, 0, 2).reshape(KC, P, NH * FOUT)
    ).astype(BF16NP)
    a_cat = np.stack(
        [a_src[..., 0].reshape(NH * FOUT), a_dst[..., 0].reshape(NH * FOUT)]
    )
    aB = np.ascontiguousarray(
        np.broadcast_to(a_cat[:, None, :], (2, P, NH * FOUT))
    ).astype(BF16NP)
    ident = np.eye(P, dtype=np.float32)
    in_maps = []
    for b in range(BS):
        hT = np.ascontiguousarray(h[b].astype(np.float32).T.reshape(KC, P, N)).astype(BF16NP)
        adjT = np.ascontiguousarray(adj[b].T.reshape(NCH, P, N)).astype(BF16NP)
        in_maps.append(
            {"hT": hT, "w_all": w_all, "aB": aB, "adjT": adjT, "ident": ident}
        )
    return in_maps


def postprocess(raw_outs):
    """raw_outs: list of [NH, FOUT+1, N] per core -> full [BS, NH, N, FOUT]."""
    outT = np.stack(raw_outs)  # [BS, NH, FOUT+1, N]
    num = outT[:, :, 0:FOUT, :]
    den = outT[:, :, FOUT : FOUT + 1, :]
    return np.ascontiguousarray((num / den).transpose(0, 1, 3, 2)).astype(np.float32)


_NC_CACHE = {}


def kernel(h, adj, w, a_src, a_dst):
    if "nc" not in _NC_CACHE:
        _NC_CACHE["nc"] = build_program(num_devices=BS)
    nc = _NC_CACHE["nc"]
    in_maps = make_in_maps(h, adj, w, a_src, a_dst)
    res = run_bass_kernel_spmd(nc, in_maps, core_ids=list(range(BS)))
    return postprocess([r["outT"] for r in res.results])


# revision 31
# speedup vs baseline: 1.0326x; 1.0011x over previous
"""Trainium2 Bass kernel for batched multi-head graph attention (GAT).

Reference computation (per batch b, head h):
    h_prime = h[b] @ w[h]                      # [N, FOUT]
    t = tanh(h_prime)
    src = t @ a_src[h]; dst = t @ a_dst[h]     # [N]
    s[i,j] = leaky_relu_{0.2}(src[i] + dst[j])
    attn = softmax_j(where(adj[b]>0, s, -inf))
    out[b,h] = attn @ h_prime

Device algorithm (core c <-> batch b=c):
    exp(leaky_relu(s)) = max(e^s, e^{0.2 s}), and with s = src_i + dst_j the
    unnormalized weight factors as
        w[j,i] = adjT[j,i] * e^{src_i} * q_j * max(1, u_i * v_j)
    with u = e^{-0.8 src}, v = e^{-0.8 dst}, q = e^{dst}. The e^{src_i} factor
    is shared by numerator and denominator of the softmax, so it cancels and is
    never computed. q_j folds into the matmul's stationary operand
    hp_q = [h_prime * q | q]; its 65th (ones*q) column accumulates the softmax
    denominator. Per 128-row chunk of the score matrix only two DVE ops run:
        mx = tensor_scalar(uB, *v_j, max 1.0)   (4x rate, bf16)
        Z  = tensor_tensor(mx, adjT, mult)      (2x rate, bf16)
    and the PE contracts outT[f,i] += hp_q[j,f] * Z[j,i]. The host divides
    rows 0..63 by row 64 and transposes to [b, h, n, f].
"""

import numpy as np
import ml_dtypes

import concourse.mybir as mybir
import concourse.tile as tile
from concourse import bacc
from concourse.bass_utils import run_bass_kernel_spmd
BS, N, FIN, NH, FOUT = 8, 1024, 256, 8, 64
P = 128
NCH = N // P          # 8 chunks of the node axis
KC = FIN // P         # 2 chunks of the feature-in axis
F32 = mybir.dt.float32
BF16 = mybir.dt.bfloat16
AX = mybir.AxisListType
ALU = mybir.AluOpType
ACTF = mybir.ActivationFunctionType
BF16NP = ml_dtypes.bfloat16

def emit(nc, tc, hT_d, w_d, aB_d, adjT_d, ident_d, out_d):
    with (
        tc.tile_pool(name="const", bufs=1) as cpool,
        tc.tile_pool(name="t", bufs=3) as tpool,
        tc.tile_pool(name="tmp", bufs=3) as mpool,
        tc.tile_pool(name="ub", bufs=3) as ubpool,
        tc.tile_pool(name="mx", bufs=6) as mxpool,
        tc.tile_pool(name="z", bufs=6) as zpool,
        tc.tile_pool(name="osb", bufs=2) as opool,
        tc.tile_pool(name="pshp", bufs=2, space="PSUM") as pp_hp,
        tc.tile_pool(name="psut", bufs=2, space="PSUM") as pp_ut,
        tc.tile_pool(name="psout", bufs=4, space="PSUM") as pp_out,
    ):
        # ---- constant loads ----
        hT = cpool.tile([P, KC, N], BF16)
        wsb = cpool.tile([P, KC, NH * FOUT], BF16)
        aB = cpool.tile([P, 2, NH * FOUT], BF16)
        adjT = cpool.tile([P, NCH, N], BF16)
        for kc in range(KC):
            for hf in range(2):
                nc.sync.dma_start(
                    hT[:, kc, hf * 512 : (hf + 1) * 512],
                    hT_d[kc, :, hf * 512 : (hf + 1) * 512],
                )
            nc.scalar.dma_start(wsb[:, kc, :], w_d[kc])
        for i in range(2):
            nc.scalar.dma_start(aB[:, i, :], aB_d[i])
        for jc in range(NCH):
            (nc.scalar if jc % 2 else nc.sync).dma_start(adjT[:, jc, :], adjT_d[jc])
        ident = cpool.tile([P, P], F32)
        nc.sync.dma_start(ident, ident_d)
        ones_row = cpool.tile([1, P], BF16)
        nc.vector.memset(ones_row, 1.0)

        hp_sb = cpool.tile([P, NCH, NH, FOUT + 1], BF16)
        nc.vector.memset(hp_sb[:, :, :, FOUT : FOUT + 1], 1.0)
        # sd_col[:, ic, 0, :] = src projection, [:, ic, 1, :] = dst projection
        sd_col = cpool.tile([P, NCH, 2, NH], F32)
        u_rows = cpool.tile([NH, N], BF16)
        uts = []

        # ---- phase A: h_prime, tanh, projections ----
        for ic in range(NCH):
            ps = pp_hp.tile([P, NH * FOUT], F32, tag="hp")
            for kc in range(KC):
                nc.tensor.matmul(
                    ps,
                    hT[:, kc, ic * P : (ic + 1) * P],
                    wsb[:, kc, :],
                    start=(kc == 0),
                    stop=(kc == KC - 1),
                )
            t = tpool.tile([P, NH, FOUT], BF16)
            nc.scalar.activation(
                t, ps.rearrange("p (h f) -> p h f", f=FOUT), ACTF.Tanh
            )
            nc.scalar.activation(
                hp_sb[:, ic, :, 0:FOUT],
                ps.rearrange("p (h f) -> p h f", f=FOUT),
                ACTF.Copy,
            )
            tm = mpool.tile([P, 2, NH, FOUT], BF16, tag="proj")
            nc.vector.tensor_tensor(
                tm,
                aB.rearrange("p w (h f) -> p w h f", f=FOUT),
                t[:, None, :, :].to_broadcast([P, 2, NH, FOUT]),
                ALU.mult,
            )
            nc.vector.tensor_reduce(sd_col[:, ic, :, :], tm, AX.X, ALU.add)
            if ic % 4 == 0:
                ut = pp_ut.tile([NH, 4 * P], F32, tag="ut", name=f"ut{ic // 4}")
                uts.append(ut)
            nc.tensor.transpose(
                uts[-1][:, (ic % 4) * P : (ic % 4 + 1) * P], sd_col[:, ic, 0, :], ident
            )
            if ic % 4 == 3:
                nc.scalar.activation(
                    u_rows[:, (ic - 3) * P : (ic + 1) * P],
                    uts[-1],
                    ACTF.Exp,
                    scale=-0.8,
                )

        # ---- phase B: exponentials ----
        q_col = cpool.tile([P, NCH, NH], F32)
        v_col = cpool.tile([P, NCH, NH], F32)
        nc.scalar.activation(q_col, sd_col[:, :, 1, :], ACTF.Exp)
        nc.scalar.activation(v_col, sd_col[:, :, 1, :], ACTF.Exp, scale=-0.8)

        # ---- phase C: masked weights + attention matmuls ----
        hp_q = cpool.tile([P, NH, NCH, FOUT + 1], BF16)
        for h in range(NH):
            for ic in range(NCH):
                if h == 0:
                    nc.vector.tensor_scalar(
                        hp_q[:, h, ic, :],
                        hp_sb[:, ic, h, :],
                        q_col[:, ic, h : h + 1],
                        None,
                        ALU.mult,
                    )
                else:
                    nc.scalar.activation(
                        hp_q[:, h, ic, :],
                        hp_sb[:, ic, h, :],
                        ACTF.Copy,
                        scale=q_col[:, ic, h : h + 1],
                    )
            if h == 0:
                row = u_rows[0:1, :]
            else:
                stage = ubpool.tile([1, N], BF16, tag="stage")
                nc.scalar.dma_start(stage, u_rows[h : h + 1, :])
                row = stage
            ub = ubpool.tile([P, N], BF16)
            for half in range(2):
                ubps = pp_ut.tile([P, 512], F32, tag="ut", name=f"ubps{half}")
                nc.tensor.matmul(
                    ubps, ones_row, row[:, half * 512 : (half + 1) * 512],
                    start=True, stop=True,
                )
                if h == 0:
                    nc.vector.tensor_copy(ub[:, half * 512 : (half + 1) * 512], ubps)
                else:
                    nc.scalar.activation(
                        ub[:, half * 512 : (half + 1) * 512], ubps, ACTF.Copy
                    )
            pso = [
                pp_out.tile([FOUT + 1, 512], F32, tag="out", name=f"pso{half}")
                for half in range(2)
            ]
            for jc2 in range(NCH // 2):
                mx = mxpool.tile([P, 2, N], BF16)
                for k in range(2):
                    jc = 2 * jc2 + k
                    nc.vector.tensor_scalar(
                        mx[:, k, :], ub, v_col[:, jc, h : h + 1], 1.0, ALU.mult, ALU.max
                    )
                z = zpool.tile([P, 2, N], BF16)
                nc.vector.tensor_tensor(
                    z, mx, adjT[:, 2 * jc2 : 2 * jc2 + 2, :], ALU.mult
                )
                for k in range(2):
                    jc = 2 * jc2 + k
                    for half in range(2):
                        nc.tensor.matmul(
                            pso[half],
                            hp_q[:, h, jc, :],
                            z[:, k, half * 512 : (half + 1) * 512],
                            start=(jc == 0),
                            stop=(jc == NCH - 1),
                        )
            ot = opool.tile([FOUT + 1, N], F32)
            nc.scalar.activation(ot[:, 0:512], pso[0], ACTF.Copy)
            nc.scalar.activation(ot[:, 512:N], pso[1], ACTF.Copy)
            nc.sync.dma_start(out_d[h], ot)


def build_program(num_devices=8, debug=False):
    nc = bacc.Bacc(
        "TRN2", target_bir_lowering=False, debug=debug, num_devices=num_devices
    )
    hT_d = nc.dram_tensor("hT", [KC, P, N], BF16, kind="ExternalInput").ap()
    w_d = nc.dram_tensor("w_all", [KC, P, NH * FOUT], BF16, kind="ExternalInput").ap()
    aB_d = nc.dram_tensor("aB", [2, P, NH * FOUT], BF16, kind="ExternalInput").ap()
    adjT_d = nc.dram_tensor("adjT", [NCH, P, N], BF16, kind="ExternalInput").ap()
    ident_d = nc.dram_tensor("ident", [P, P], F32, kind="ExternalInput").ap()
    out_d = nc.dram_tensor("outT", [NH, FOUT + 1, N], F32, kind="ExternalOutput").ap()
    with tile.TileContext(nc) as tc:
        emit(nc, tc, hT_d, w_d, aB_d, adjT_d, ident_d, out_d)
    nc.compile()
    return nc


def make_in_maps(h, adj, w, a_src, a_dst):
    """Host-side sharding/layout prep: core c gets batch c."""
    w_all = np.ascontiguousarray(
        w.astype(np.float32).transpose(1, 0, 2).reshape(KC, P, NH * FOUT)
    ).astype(BF16NP)
    a_cat = np.stack(
        [a_src[..., 0].reshape(NH * FOUT), a_dst[..., 0].reshape(NH * FOUT)]
    )
    aB = np.ascontiguousarray(
        np.broadcast_to(a_cat[:, None, :], (2, P, NH * FOUT))
    ).astype(BF16NP)
    ident = np.eye(P, dtype=np.float32)
    in_maps = []
    for b in range(BS):
        hT = np.ascontiguousarray(h[b].astype(np.float32).T.reshape(KC, P, N)).astype(BF16NP)
        adjT = np.ascontiguousarray(adj[b].T.reshape(NCH, P, N)).astype(BF16NP)
        in_maps.append(
            {"hT": hT, "w_all": w_all, "aB": aB, "adjT": adjT, "ident": ident}
        )
    return in_maps


def postprocess(raw_outs):
    """raw_outs: list of [NH, FOUT+1, N] per core -> full [BS, NH, N, FOUT]."""
    outT = np.stack(raw_outs)  # [BS, NH, FOUT+1, N]
    num = outT[:, :, 0:FOUT, :]
    den = outT[:, :, FOUT : FOUT + 1, :]
    return np.ascontiguousarray((num / den).transpose(0, 1, 3, 2)).astype(np.float32)


_NC_CACHE = {}


def kernel(h, adj, w, a_src, a_dst):
    if "nc" not in _NC_CACHE:
        _NC_CACHE["nc"] = build_program(num_devices=BS)
    nc = _NC_CACHE["nc"]
    in_maps = make_in_maps(h, adj, w, a_src, a_dst)
    res = run_bass_kernel_spmd(nc, in_maps, core_ids=list(range(BS)))
    return postprocess([r["outT"] for r in res.results])
